# revision 12
# baseline (speedup 1.0000x reference)
"""Ball attention (block-local attention, ball size 128) on 8 Trainium2 cores.

Reference computation (per (b,h) head, per ball of 128 consecutive tokens):
    S = Q K^T / sqrt(64);  P = softmax(S, axis=-1);  O = P V

Sharding: the 64 (b,h) heads are split 8-per-core (pure data parallel).
The host shard step stages each core's inputs in a DMA-friendly tiling
[head, token-in-ball, ball, d] (a pure byte reorder of the same fp32
values; the gather step applies the inverse to the output). V is staged
with a 65th ones column so softmax denominators fall out of the O matmul.

Why: with the natural [head, seq, d] layout, every (partition, ball) pair
is a separate 256-byte DRAM run, so the 64 MiB/core of HBM traffic costs
~262k DMA descriptors at ~18 ns each — measured 84-91% SDMA busy and the
dominant cost. The ball-major tiling makes per-partition runs 16 KiB
(~4k descriptors total) so DMA runs at payload rate.

Per-core compute (HW-measured on the fp32/fp32r baseline and bf16 v2):
  * Loads via SWDGE (gpsimd) DMA with inline fp32->bf16 cast.
  * Q^T/K^T per ball pair as plain matmuls (stationary = 2-ball packed
    [128 seq, 2x64d] bf16 slab, moving = bf16 identity): out fp32 PSUM
    [2x64d, 128 seq]. Plain MM streams at 1 cyc/col vs transpose-mode's
    1.2 GHz path (measured 107ns -> ~55ns/op). PSUM->SBUF copies round
    to bf16: DVE takes Q^T, ACT takes K^T.
  * S^T = K Q^T per ball: bf16 matmul N=128, contraction 64 rows at base
    partition 64*(ball parity): consecutive matmuls hit disjoint row
    halves and overlap in the PE array (measured ~4ns second-of-pair);
    PSUM bank alternates with parity (concurrent same-bank writes fault).
  * E = exp(S^T/8): one ACT op per 4-ball group, bf16 out, slot (b2,a2).
  * O_unnorm = E^T [V|1]: bf16 matmuls N=65 (measured ~54ns/op).
  * Normalize on DVE via per-partition reciprocal broadcast; store fp32
    in the ball-major tiling on the SP HWDGE ring.
"""

import os
import sys

for _p in ("/opt/trn_rl_repo",):
    if _p not in sys.path and os.path.isdir(_p):
        sys.path.insert(0, _p)

from contextlib import ExitStack

import numpy as np

import concourse.bass as bass
import concourse.mybir as mybir
import concourse.tile as tile
from concourse import bacc
from concourse._compat import with_exitstack
from concourse.masks import make_identity

B, H, N, DH = 4, 16, 8192, 64
BS = 128                 # ball size == SBUF partition count
NCORES = 8
HEADS = B * H // NCORES  # heads per core (8)
M = N // BS              # balls per head (64)

FP32 = mybir.dt.float32
BF16 = mybir.dt.bfloat16

GRP = 4
NCHUNK = int(os.environ.get("BALL_NCHUNK", "2"))  # head-load split


@with_exitstack
def ball_attention_kernel(
    ctx: ExitStack,
    tc: tile.TileContext,
    out_ap: bass.AP,
    q_ap: bass.AP,
    k_ap: bass.AP,
    v_ap: bass.AP,
    heads: int = HEADS,
    m: int = M,
):
    nc = tc.nc
    assert m % GRP == 0
    ngrp = m // GRP
    scale = 1.0 / float(np.sqrt(DH))

    const_pool = ctx.enter_context(tc.tile_pool(name="const", bufs=1))
    io_pool = ctx.enter_context(tc.tile_pool(name="io", bufs=3))
    t_sb_pool = ctx.enter_context(tc.tile_pool(name="t_sb", bufs=3))
    e_pool = ctx.enter_context(tc.tile_pool(name="e", bufs=2))
    r_pool = ctx.enter_context(tc.tile_pool(name="r", bufs=2))
    t_ps_pool = ctx.enter_context(tc.tile_pool(name="t_ps", bufs=2, space="PSUM"))
    s_ps_pool = ctx.enter_context(tc.tile_pool(name="s_ps", bufs=2, space="PSUM"))
    o_ps_pool = ctx.enter_context(tc.tile_pool(name="o_ps", bufs=2, space="PSUM"))

    ident = const_pool.tile([BS, BS], BF16)

    for h in range(heads):
        # ---- loads: SWDGE casts fp32 -> bf16; ball-major staging means the
        # per-partition DRAM run is a whole [ball, d] row (16 KiB).
        # The last head loads in fine chunks: its compute pipelines with the
        # final stretch of the load stream instead of waiting for the whole
        # half-head, shrinking the compute-paced tail after the last byte.
        nch = 8 if h == heads - 1 else NCHUNK
        mc = m // nch
        q_sb = io_pool.tile([BS, m, DH], BF16, tag="q")
        k_sb = io_pool.tile([BS, m, DH], BF16, tag="k")
        vt = io_pool.tile([BS, m, DH + 1], BF16, tag="vt")
        qv_ = q_ap[h].rearrange("(p mm) d -> p mm d", p=BS)
        kv_ = k_ap[h].rearrange("(p mm) d -> p mm d", p=BS)
        vv_ = v_ap[h]  # already [BS, m, DH+1] with the host-staged ones col
        for c in range(nch):
            cs = slice(c * mc, (c + 1) * mc)
            nc.gpsimd.dma_start(q_sb[:, cs, :], qv_[:, cs, :])
            nc.gpsimd.dma_start(k_sb[:, cs, :], kv_[:, cs, :])
            nc.gpsimd.dma_start(vt[:, cs, :], vv_[:, cs, :])
        if h == 0:
            # after the first load burst: Q7 starts descgen immediately and
            # the identity is still ready long before the first transpose.
            make_identity(nc, ident)
        ob = io_pool.tile([BS, m, DH], FP32, tag="ob")

        for g in range(ngrp):
            m0 = g * GRP
            # ---- transposes: packed 2-ball plain matmuls, fp32 PSUM out --
            t_ps = t_ps_pool.tile([BS, 4, BS], FP32, tag="t")
            nc.tensor.matmul(t_ps[:, 0, :], q_sb[:, m0 : m0 + 2, :], ident, start=True, stop=True)
            nc.tensor.matmul(t_ps[:, 1, :], k_sb[:, m0 : m0 + 2, :], ident, start=True, stop=True)
            nc.tensor.matmul(t_ps[:, 2, :], q_sb[:, m0 + 2 : m0 + 4, :], ident, start=True, stop=True)
            nc.tensor.matmul(t_ps[:, 3, :], k_sb[:, m0 + 2 : m0 + 4, :], ident, start=True, stop=True)
            qt = t_sb_pool.tile([BS, 2, BS], BF16, tag="qt")  # [pair, seq]
            kt = t_sb_pool.tile([BS, 2, BS], BF16, tag="kt")
            nc.vector.tensor_copy(qt, t_ps[:, 0:4:2, :])
            nc.scalar.copy(kt, t_ps[:, 1:4:2, :])

            # ---- S^T matmuls: per-ball N=128 bf16 ------------------------
            # ball m0+j: pair a2 = j>>1, parity b = j&1; operands live on
            # partitions [64b, 64b+64) -> consecutive matmuls hit different
            # row halves and run concurrently, so they must also hit
            # different PSUM banks: dim 1 of s_ps strides a full 2 KiB bank.
            s_ps = s_ps_pool.tile([BS, 2, 2, 2, BS], FP32, tag="s")
            for j in range(GRP):
                a2, b = j >> 1, j & 1
                lo = 64 * b
                nc.tensor.matmul(
                    s_ps[:, b, a2, 0, :],
                    kt[lo : lo + 64, a2, :],
                    qt[lo : lo + 64, a2, :],
                    start=True,
                    stop=True,
                )
            # ---- E = exp(S^T/8): one ACT op per group, bf16, slot (b,a2) -
            e_sb = e_pool.tile([BS, 2, 2, BS], BF16, tag="e")
            nc.scalar.activation(
                e_sb,
                s_ps[:, :, :, 0, :],
                mybir.ActivationFunctionType.Exp,
                scale=scale,
            )

            # ---- O_unnorm = E^T @ [V | 1] --------------------------------
            o_ps = o_ps_pool.tile([BS, GRP, DH + 1], FP32, tag="o")
            for j in range(GRP):
                a2, b = j >> 1, j & 1
                nc.tensor.matmul(
                    o_ps[:, j, :],
                    e_sb[:, b, a2, :],
                    vt[:, m0 + j, :],
                    start=True,
                    stop=True,
                )
            # ---- normalize by the ones-column sums -----------------------
            r_sb = r_pool.tile([BS, GRP], FP32, tag="r")
            nc.vector.reciprocal(r_sb, o_ps[:, :, DH])
            nc.vector.tensor_mul(
                ob[:, m0 : m0 + GRP, :],
                o_ps[:, :, 0:DH],
                r_sb.unsqueeze(2).broadcast_to([BS, GRP, DH]),
            )

            # ---- store every 4 groups so the output drains continuously --
            if g % 4 == 3:
                ms = slice(m0 + GRP - 16, m0 + GRP)
                nc.sync.dma_start(
                    out_ap[h].rearrange("(p mm) d -> p mm d", p=BS)[:, ms, :],
                    ob[:, ms, :],
                )


def build_nc(heads: int = HEADS, m: int = M):
    nc = bacc.Bacc("TRN2", target_bir_lowering=False, debug=False, num_devices=NCORES)
    q = nc.dram_tensor("q", [heads, m * BS, DH], FP32, kind="ExternalInput").ap()
    k = nc.dram_tensor("k", [heads, m * BS, DH], FP32, kind="ExternalInput").ap()
    v = nc.dram_tensor("v", [heads, BS, M, DH + 1], FP32, kind="ExternalInput").ap()
    o = nc.dram_tensor("out", [heads, m * BS, DH], FP32, kind="ExternalOutput").ap()
    with tile.TileContext(nc) as tc:
        ball_attention_kernel(tc, o, q, k, v, heads=heads, m=m)
    nc.compile()
    return nc


_NC_CACHE = {}


def _stage_qk(x: np.ndarray) -> np.ndarray:
    """[heads, N, DH] fp32 -> ball-major [heads, BS*M, DH] (token, ball, d)."""
    hp = x.shape[0]
    return np.ascontiguousarray(
        x.reshape(hp, M, BS, DH).transpose(0, 2, 1, 3).reshape(hp, N, DH)
    )


def _stage_v(x: np.ndarray) -> np.ndarray:
    """[heads, N, DH] fp32 -> ball-major [heads, BS, M, DH+1] with ones col."""
    hp = x.shape[0]
    out = np.empty((hp, BS, M, DH + 1), dtype=np.float32)
    out[..., :DH] = x.reshape(hp, M, BS, DH).transpose(0, 2, 1, 3)
    out[..., DH] = 1.0
    return out


def kernel(q: np.ndarray, k: np.ndarray, v: np.ndarray) -> np.ndarray:
    from concourse.bass_utils import run_bass_kernel_spmd

    assert q.shape == (B, H, N, DH)
    if "nc" not in _NC_CACHE:
        _NC_CACHE["nc"] = build_nc()
    nc = _NC_CACHE["nc"]

    hpc = HEADS
    qf = np.asarray(q, dtype=np.float32).reshape(B * H, N, DH)
    kf = np.asarray(k, dtype=np.float32).reshape(B * H, N, DH)
    vf = np.asarray(v, dtype=np.float32).reshape(B * H, N, DH)
    in_maps = [
        {
            "q": _stage_qk(qf[c * hpc : (c + 1) * hpc]),
            "k": _stage_qk(kf[c * hpc : (c + 1) * hpc]),
            "v": _stage_v(vf[c * hpc : (c + 1) * hpc]),
        }
        for c in range(NCORES)
    ]
    res = run_bass_kernel_spmd(nc, in_maps, core_ids=list(range(NCORES)))
    out = np.concatenate([res.results[c]["out"] for c in range(NCORES)], axis=0)
    # un-permute: device wrote [head, token-in-ball, ball, d]
    out = out.reshape(B * H, BS, M, DH).transpose(0, 2, 1, 3)
    return np.ascontiguousarray(out).reshape(B, H, N, DH)


# revision 13
# speedup vs baseline: 1.0760x; 1.0760x over previous
"""Ball attention (block-local attention, ball size 128) on 8 Trainium2 cores.

Reference computation (per (b,h) head, per ball of 128 consecutive tokens):
    S = Q K^T / sqrt(64);  P = softmax(S, axis=-1);  O = P V

Sharding: the 64 (b,h) heads are split 8-per-core (pure data parallel).
The host shard step stages each core's inputs in a DMA-friendly tiling
[head, token-in-ball, ball, d] (a pure byte reorder of the same fp32
values; the gather step applies the inverse to the output). V is staged
with a 65th ones column so softmax denominators fall out of the O matmul.

Why: with the natural [head, seq, d] layout, every (partition, ball) pair
is a separate 256-byte DRAM run, so the 64 MiB/core of HBM traffic costs
~262k DMA descriptors at ~18 ns each — measured 84-91% SDMA busy and the
dominant cost. The ball-major tiling makes per-partition runs 16 KiB
(~4k descriptors total) so DMA runs at payload rate.

Per-core compute (HW-measured on the fp32/fp32r baseline and bf16 v2):
  * Loads via SWDGE (gpsimd) DMA with inline fp32->bf16 cast.
  * Q^T/K^T per ball pair as plain matmuls (stationary = 2-ball packed
    [128 seq, 2x64d] bf16 slab, moving = bf16 identity): out fp32 PSUM
    [2x64d, 128 seq]. Plain MM streams at 1 cyc/col vs transpose-mode's
    1.2 GHz path (measured 107ns -> ~55ns/op). PSUM->SBUF copies round
    to bf16: DVE takes Q^T, ACT takes K^T.
  * S^T = K Q^T per ball: bf16 matmul N=128, contraction 64 rows at base
    partition 64*(ball parity): consecutive matmuls hit disjoint row
    halves and overlap in the PE array (measured ~4ns second-of-pair);
    PSUM bank alternates with parity (concurrent same-bank writes fault).
  * E = exp(S^T/8): one ACT op per 4-ball group, bf16 out, slot (b2,a2).
  * O_unnorm = E^T [V|1]: bf16 matmuls N=65 (measured ~54ns/op).
  * Normalize on DVE via per-partition reciprocal broadcast; store fp32
    in the ball-major tiling on the SP HWDGE ring, a quarter-head at a
    time so the write stream drains continuously.

Measured timeline (fast run, 193us total): ~9us ramp (preamble + first
Q7 descgen), ~174us DMA-paced stream (SDMA >90% busy at ~356 GB/s ~= the
358 GB/s HBM-per-NC roofline for the 64.5 MiB/core of fp32 I/O), ~5us
compute tail (last head loads in 8 fine chunks so only ~2 groups of work
remain after the final bytes land), ~3us final store, ~8.5us framework
epilogue. HW exec time over repeated runs: 193-221us (device has
fast/slow phases); baseline was 363us. PE runs at 1.2 GHz throughout
(never observed HAM-warm 2.4 GHz); PE busy ~153us hides under the DMA.
"""

import os
import sys

for _p in ("/opt/trn_rl_repo",):
    if _p not in sys.path and os.path.isdir(_p):
        sys.path.insert(0, _p)

from contextlib import ExitStack

import numpy as np

import concourse.bass as bass
import concourse.mybir as mybir
import concourse.tile as tile
from concourse import bacc
from concourse._compat import with_exitstack
from concourse.masks import make_identity

B, H, N, DH = 4, 16, 8192, 64
BS = 128                 # ball size == SBUF partition count
NCORES = 8
HEADS = B * H // NCORES  # heads per core (8)
M = N // BS              # balls per head (64)

FP32 = mybir.dt.float32
BF16 = mybir.dt.bfloat16

GRP = 4
NCHUNK = int(os.environ.get("BALL_NCHUNK", "2"))  # head-load split


@with_exitstack
def ball_attention_kernel(
    ctx: ExitStack,
    tc: tile.TileContext,
    out_ap: bass.AP,
    q_ap: bass.AP,
    k_ap: bass.AP,
    v_ap: bass.AP,
    heads: int = HEADS,
    m: int = M,
):
    nc = tc.nc
    assert m % GRP == 0
    ngrp = m // GRP
    scale = 1.0 / float(np.sqrt(DH))

    const_pool = ctx.enter_context(tc.tile_pool(name="const", bufs=1))
    io_pool = ctx.enter_context(tc.tile_pool(name="io", bufs=3))
    t_sb_pool = ctx.enter_context(tc.tile_pool(name="t_sb", bufs=3))
    e_pool = ctx.enter_context(tc.tile_pool(name="e", bufs=2))
    r_pool = ctx.enter_context(tc.tile_pool(name="r", bufs=2))
    t_ps_pool = ctx.enter_context(tc.tile_pool(name="t_ps", bufs=2, space="PSUM"))
    s_ps_pool = ctx.enter_context(tc.tile_pool(name="s_ps", bufs=2, space="PSUM"))
    o_ps_pool = ctx.enter_context(tc.tile_pool(name="o_ps", bufs=2, space="PSUM"))

    ident = const_pool.tile([BS, BS], BF16)

    for h in range(heads):
        # ---- loads: SWDGE casts fp32 -> bf16; ball-major staging means the
        # per-partition DRAM run is a whole [ball, d] row (16 KiB).
        # The last head loads in fine chunks: its compute pipelines with the
        # final stretch of the load stream instead of waiting for the whole
        # half-head, shrinking the compute-paced tail after the last byte.
        nch = 8 if h == heads - 1 else NCHUNK
        mc = m // nch
        q_sb = io_pool.tile([BS, m, DH], BF16, tag="q")
        k_sb = io_pool.tile([BS, m, DH], BF16, tag="k")
        vt = io_pool.tile([BS, m, DH + 1], BF16, tag="vt")
        qv_ = q_ap[h].rearrange("(p mm) d -> p mm d", p=BS)
        kv_ = k_ap[h].rearrange("(p mm) d -> p mm d", p=BS)
        vv_ = v_ap[h]  # already [BS, m, DH+1] with the host-staged ones col
        for c in range(nch):
            cs = slice(c * mc, (c + 1) * mc)
            nc.gpsimd.dma_start(q_sb[:, cs, :], qv_[:, cs, :])
            nc.gpsimd.dma_start(k_sb[:, cs, :], kv_[:, cs, :])
            nc.gpsimd.dma_start(vt[:, cs, :], vv_[:, cs, :])
        if h == 0:
            # after the first load burst: Q7 starts descgen immediately and
            # the identity is still ready long before the first transpose.
            make_identity(nc, ident)
        ob = io_pool.tile([BS, m, DH], FP32, tag="ob")

        for g in range(ngrp):
            m0 = g * GRP
            # ---- transposes: packed 2-ball plain matmuls, fp32 PSUM out --
            t_ps = t_ps_pool.tile([BS, 4, BS], FP32, tag="t")
            nc.tensor.matmul(t_ps[:, 0, :], q_sb[:, m0 : m0 + 2, :], ident, start=True, stop=True)
            nc.tensor.matmul(t_ps[:, 1, :], k_sb[:, m0 : m0 + 2, :], ident, start=True, stop=True)
            nc.tensor.matmul(t_ps[:, 2, :], q_sb[:, m0 + 2 : m0 + 4, :], ident, start=True, stop=True)
            nc.tensor.matmul(t_ps[:, 3, :], k_sb[:, m0 + 2 : m0 + 4, :], ident, start=True, stop=True)
            qt = t_sb_pool.tile([BS, 2, BS], BF16, tag="qt")  # [pair, seq]
            kt = t_sb_pool.tile([BS, 2, BS], BF16, tag="kt")
            nc.vector.tensor_copy(qt, t_ps[:, 0:4:2, :])
            nc.scalar.copy(kt, t_ps[:, 1:4:2, :])

            # ---- S^T matmuls: per-ball N=128 bf16 ------------------------
            # ball m0+j: pair a2 = j>>1, parity b = j&1; operands live on
            # partitions [64b, 64b+64) -> consecutive matmuls hit different
            # row halves and run concurrently, so they must also hit
            # different PSUM banks: dim 1 of s_ps strides a full 2 KiB bank.
            s_ps = s_ps_pool.tile([BS, 2, 2, 2, BS], FP32, tag="s")
            for j in range(GRP):
                a2, b = j >> 1, j & 1
                lo = 64 * b
                nc.tensor.matmul(
                    s_ps[:, b, a2, 0, :],
                    kt[lo : lo + 64, a2, :],
                    qt[lo : lo + 64, a2, :],
                    start=True,
                    stop=True,
                )
            # ---- E = exp(S^T/8): one ACT op per group, bf16, slot (b,a2) -
            e_sb = e_pool.tile([BS, 2, 2, BS], BF16, tag="e")
            nc.scalar.activation(
                e_sb,
                s_ps[:, :, :, 0, :],
                mybir.ActivationFunctionType.Exp,
                scale=scale,
            )

            # ---- O_unnorm = E^T @ [V | 1] --------------------------------
            o_ps = o_ps_pool.tile([BS, GRP, DH + 1], FP32, tag="o")
            for j in range(GRP):
                a2, b = j >> 1, j & 1
                nc.tensor.matmul(
                    o_ps[:, j, :],
                    e_sb[:, b, a2, :],
                    vt[:, m0 + j, :],
                    start=True,
                    stop=True,
                )
            # ---- normalize by the ones-column sums -----------------------
            r_sb = r_pool.tile([BS, GRP], FP32, tag="r")
            nc.vector.reciprocal(r_sb, o_ps[:, :, DH])
            nc.vector.tensor_mul(
                ob[:, m0 : m0 + GRP, :],
                o_ps[:, :, 0:DH],
                r_sb.unsqueeze(2).broadcast_to([BS, GRP, DH]),
            )

            # ---- store every 4 groups so the output drains continuously --
            if g % 4 == 3:
                ms = slice(m0 + GRP - 16, m0 + GRP)
                nc.sync.dma_start(
                    out_ap[h].rearrange("(p mm) d -> p mm d", p=BS)[:, ms, :],
                    ob[:, ms, :],
                )


def build_nc(heads: int = HEADS, m: int = M):
    nc = bacc.Bacc("TRN2", target_bir_lowering=False, debug=False, num_devices=NCORES)
    q = nc.dram_tensor("q", [heads, m * BS, DH], FP32, kind="ExternalInput").ap()
    k = nc.dram_tensor("k", [heads, m * BS, DH], FP32, kind="ExternalInput").ap()
    v = nc.dram_tensor("v", [heads, BS, M, DH + 1], FP32, kind="ExternalInput").ap()
    o = nc.dram_tensor("out", [heads, m * BS, DH], FP32, kind="ExternalOutput").ap()
    with tile.TileContext(nc) as tc:
        ball_attention_kernel(tc, o, q, k, v, heads=heads, m=m)
    nc.compile()
    return nc


_NC_CACHE = {}


def _stage_qk(x: np.ndarray) -> np.ndarray:
    """[heads, N, DH] fp32 -> ball-major [heads, BS*M, DH] (token, ball, d)."""
    hp = x.shape[0]
    return np.ascontiguousarray(
        x.reshape(hp, M, BS, DH).transpose(0, 2, 1, 3).reshape(hp, N, DH)
    )


def _stage_v(x: np.ndarray) -> np.ndarray:
    """[heads, N, DH] fp32 -> ball-major [heads, BS, M, DH+1] with ones col."""
    hp = x.shape[0]
    out = np.empty((hp, BS, M, DH + 1), dtype=np.float32)
    out[..., :DH] = x.reshape(hp, M, BS, DH).transpose(0, 2, 1, 3)
    out[..., DH] = 1.0
    return out


def kernel(q: np.ndarray, k: np.ndarray, v: np.ndarray) -> np.ndarray:
    from concourse.bass_utils import run_bass_kernel_spmd

    assert q.shape == (B, H, N, DH)
    if "nc" not in _NC_CACHE:
        _NC_CACHE["nc"] = build_nc()
    nc = _NC_CACHE["nc"]

    hpc = HEADS
    qf = np.asarray(q, dtype=np.float32).reshape(B * H, N, DH)
    kf = np.asarray(k, dtype=np.float32).reshape(B * H, N, DH)
    vf = np.asarray(v, dtype=np.float32).reshape(B * H, N, DH)
    in_maps = [
        {
            "q": _stage_qk(qf[c * hpc : (c + 1) * hpc]),
            "k": _stage_qk(kf[c * hpc : (c + 1) * hpc]),
            "v": _stage_v(vf[c * hpc : (c + 1) * hpc]),
        }
        for c in range(NCORES)
    ]
    res = run_bass_kernel_spmd(nc, in_maps, core_ids=list(range(NCORES)))
    out = np.concatenate([res.results[c]["out"] for c in range(NCORES)], axis=0)
    # un-permute: device wrote [head, token-in-ball, ball, d]
    out = out.reshape(B * H, BS, M, DH).transpose(0, 2, 1, 3)
    return np.ascontiguousarray(out).reshape(B, H, N, DH)
